# revision 1
# baseline (speedup 1.0000x reference)
"""Sliding-window KV-cache append kernel for Trainium2 (8 NeuronCores).

Reference semantics (per tensor, f32):
    out = concat([cache, new], axis=2)[:, :, -MAX_LEN:, :]
which is a pure shift-and-append:
    out[:, :, :MAX_LEN-NEW, :] = cache[:, :, NEW:, :]
    out[:, :, MAX_LEN-NEW:, :] = new

Sharding: flatten (B, H) -> BH=128 and split across 8 cores (16 slices each).
The seq axis stays local, so per core the whole job is 4 DRAM->DRAM DMAs:
bulk shifted-cache copy + new-token append, for k and for v.
"""

import sys

for _p in ("/opt/trn_rl_repo",):
    if _p not in sys.path:
        sys.path.insert(0, _p)

import numpy as np

B, H, MAX_LEN, D = 4, 32, 4096, 128
NEW = 16
KEEP = MAX_LEN - NEW  # 4080
N_CORES = 8
BH = B * H  # 128
SH = BH // N_CORES  # 16 slices per core

_nc_cache = {}


def _build_bass():
    import concourse.bass as bass
    import concourse.mybir as mybir

    nc = bass.Bass(trn_type="TRN2")
    f32 = mybir.dt.float32
    ck = nc.dram_tensor("cache_k", [SH, MAX_LEN, D], f32, kind="ExternalInput")
    cv = nc.dram_tensor("cache_v", [SH, MAX_LEN, D], f32, kind="ExternalInput")
    kn = nc.dram_tensor("k", [SH, NEW, D], f32, kind="ExternalInput")
    vn = nc.dram_tensor("v", [SH, NEW, D], f32, kind="ExternalInput")
    ok = nc.dram_tensor("out_k", [SH, MAX_LEN, D], f32, kind="ExternalOutput")
    ov = nc.dram_tensor("out_v", [SH, MAX_LEN, D], f32, kind="ExternalOutput")

    with nc.semaphore("dma_sem") as sem, nc.Block() as block:

        @block.sync
        def _(sync):
            sync.dma_start(out=ok[:, :KEEP, :], in_=ck[:, NEW:, :]).then_inc(sem, 16)
            sync.dma_start(out=ov[:, :KEEP, :], in_=cv[:, NEW:, :]).then_inc(sem, 16)
            sync.dma_start(out=ok[:, KEEP:, :], in_=kn[:, :, :]).then_inc(sem, 16)
            sync.dma_start(out=ov[:, KEEP:, :], in_=vn[:, :, :]).then_inc(sem, 16)
            sync.wait_ge(sem, 64)

    return nc


def _get_nc():
    if "nc" not in _nc_cache:
        _nc_cache["nc"] = _build_bass()
    return _nc_cache["nc"]


def _run(inputs_by_core, trace=False, **kw):
    from concourse import bass_utils

    nc = _get_nc()
    return bass_utils.run_bass_kernel_spmd(
        nc, inputs_by_core, core_ids=list(range(N_CORES)), trace=trace, **kw
    )


def kernel(cache_k, cache_v, k, v, _trace=False, _ret_perf=False, **_kw):
    cache_k = np.ascontiguousarray(np.asarray(cache_k, dtype=np.float32)).reshape(
        BH, MAX_LEN, D
    )
    cache_v = np.ascontiguousarray(np.asarray(cache_v, dtype=np.float32)).reshape(
        BH, MAX_LEN, D
    )
    k = np.ascontiguousarray(np.asarray(k, dtype=np.float32)).reshape(BH, NEW, D)
    v = np.ascontiguousarray(np.asarray(v, dtype=np.float32)).reshape(BH, NEW, D)

    in_maps = []
    for c in range(N_CORES):
        s = slice(c * SH, (c + 1) * SH)
        in_maps.append(
            {
                "cache_k": cache_k[s],
                "cache_v": cache_v[s],
                "k": k[s],
                "v": v[s],
            }
        )

    res = _run(in_maps, trace=_trace, **_kw)

    out_k = np.concatenate([r["out_k"] for r in res.results], axis=0).reshape(
        B, H, MAX_LEN, D
    )
    out_v = np.concatenate([r["out_v"] for r in res.results], axis=0).reshape(
        B, H, MAX_LEN, D
    )
    if _ret_perf:
        return (out_k, out_v), res
    return (out_k, out_v)


# revision 9
# speedup vs baseline: 1.5354x; 1.5354x over previous
"""Sliding-window KV-cache append kernel for Trainium2 (8 NeuronCores).

Reference semantics (per tensor, f32):
    out = concat([cache, new], axis=2)[:, :, -MAX_LEN:, :]
which is a pure shift-and-append:
    out[:, :, :MAX_LEN-NEW, :] = cache[:, :, NEW:, :]
    out[:, :, MAX_LEN-NEW:, :] = new

Sharding: flatten (B, H) -> BH=128 and split across 8 cores (16 slices each).
The seq axis stays local, so per core the whole job is a handful of
DRAM->DRAM DMAs: bulk shifted-cache copy + new-token append, for k and v.
"""

import sys

for _p in ("/opt/trn_rl_repo",):
    if _p not in sys.path:
        sys.path.insert(0, _p)

import numpy as np

B, H, MAX_LEN, D = 4, 32, 4096, 128
NEW = 16
KEEP = MAX_LEN - NEW  # 4080
N_CORES = 8
BH = B * H  # 128
SH = BH // N_CORES  # 16 slices per core

# Fastest measured variant: bulk work split across the three descriptor-
# generation paths (sync HWDGE, scalar HWDGE, gpsimd SWDGE), full-slice
# per-engine streams, with a tiny trailing DMA per HWDGE queue so the final
# completion receipt is short. Sustains ~275-282 GB/s of move (~550-565 GB/s
# HBM traffic) per NeuronCore -- the measured per-NC ceiling.
VARIANT = "tr_tail"

_nc_cache = {}


def _build_bass(variant):
    import concourse.bass as bass
    import concourse.mybir as mybir

    nc = bass.Bass(trn_type="TRN2")
    f32 = mybir.dt.float32

    if variant == "merged":
        ckv = nc.dram_tensor("cache_kv", [2 * SH, MAX_LEN, D], f32, kind="ExternalInput")
        kvn = nc.dram_tensor("kv_new", [2 * SH, NEW, D], f32, kind="ExternalInput")
        okv = nc.dram_tensor("out_kv", [2 * SH, MAX_LEN, D], f32, kind="ExternalOutput")
        with nc.semaphore("dma_sem") as sem, nc.Block() as block:

            @block.sync
            def _(sync):
                sync.dma_start(out=okv[:, KEEP:, :], in_=kvn[:, :, :]).then_inc(sem, 16)
                sync.dma_start(out=okv[:, :KEEP, :], in_=ckv[:, NEW:, :]).then_inc(sem, 16)
                sync.wait_ge(sem, 32)

        return nc

    ck = nc.dram_tensor("cache_k", [SH, MAX_LEN, D], f32, kind="ExternalInput")
    cv = nc.dram_tensor("cache_v", [SH, MAX_LEN, D], f32, kind="ExternalInput")
    kn = nc.dram_tensor("k", [SH, NEW, D], f32, kind="ExternalInput")
    vn = nc.dram_tensor("v", [SH, NEW, D], f32, kind="ExternalInput")
    ok = nc.dram_tensor("out_k", [SH, MAX_LEN, D], f32, kind="ExternalOutput")
    ov = nc.dram_tensor("out_v", [SH, MAX_LEN, D], f32, kind="ExternalOutput")

    if variant == "base":
        with nc.semaphore("dma_sem") as sem, nc.Block() as block:

            @block.sync
            def _(sync):
                sync.dma_start(out=ok[:, :KEEP, :], in_=ck[:, NEW:, :]).then_inc(sem, 16)
                sync.dma_start(out=ov[:, :KEEP, :], in_=cv[:, NEW:, :]).then_inc(sem, 16)
                sync.dma_start(out=ok[:, KEEP:, :], in_=kn[:, :, :]).then_inc(sem, 16)
                sync.dma_start(out=ov[:, KEEP:, :], in_=vn[:, :, :]).then_inc(sem, 16)
                sync.wait_ge(sem, 64)

    elif variant == "small_first":
        with nc.semaphore("dma_sem") as sem, nc.Block() as block:

            @block.sync
            def _(sync):
                sync.dma_start(out=ok[:, KEEP:, :], in_=kn[:, :, :]).then_inc(sem, 16)
                sync.dma_start(out=ov[:, KEEP:, :], in_=vn[:, :, :]).then_inc(sem, 16)
                sync.dma_start(out=ok[:, :KEEP, :], in_=ck[:, NEW:, :]).then_inc(sem, 16)
                sync.dma_start(out=ov[:, :KEEP, :], in_=cv[:, NEW:, :]).then_inc(sem, 16)
                sync.wait_ge(sem, 64)

    elif variant == "two_rings":
        with (
            nc.semaphore("dma_sem_k") as sem_k,
            nc.semaphore("dma_sem_v") as sem_v,
            nc.Block() as block,
        ):

            @block.sync
            def _(sync):
                sync.dma_start(out=ok[:, KEEP:, :], in_=kn[:, :, :]).then_inc(sem_k, 16)
                sync.dma_start(out=ok[:, :KEEP, :], in_=ck[:, NEW:, :]).then_inc(sem_k, 16)
                sync.wait_ge(sem_k, 32)
                sync.wait_ge(sem_v, 32)

            @block.scalar
            def _(scalar):
                scalar.dma_start(out=ov[:, KEEP:, :], in_=vn[:, :, :]).then_inc(sem_v, 16)
                scalar.dma_start(out=ov[:, :KEEP, :], in_=cv[:, NEW:, :]).then_inc(sem_v, 16)

    elif variant == "three_rings":
        # Split the 64 MiB of bulk work ~evenly over both HWDGE rings
        # (sync=SP, scalar=ACT) and the SWDGE (gpsimd) path.
        with (
            nc.semaphore("sem_a") as sem_a,
            nc.semaphore("sem_b") as sem_b,
            nc.semaphore("sem_c") as sem_c,
            nc.Block() as block,
        ):

            @block.sync
            def _(sync):
                sync.dma_start(out=ok[:, KEEP:, :], in_=kn[:, :, :]).then_inc(sem_a, 16)
                sync.dma_start(
                    out=ok[:11, :KEEP, :], in_=ck[:11, NEW:, :]
                ).then_inc(sem_a, 16)
                sync.wait_ge(sem_a, 32)
                sync.wait_ge(sem_b, 32)
                sync.wait_ge(sem_c, 32)

            @block.scalar
            def _(scalar):
                scalar.dma_start(out=ov[:, KEEP:, :], in_=vn[:, :, :]).then_inc(
                    sem_b, 16
                )
                scalar.dma_start(
                    out=ov[:11, :KEEP, :], in_=cv[:11, NEW:, :]
                ).then_inc(sem_b, 16)

            @block.gpsimd
            def _(gpsimd):
                gpsimd.dma_start(
                    out=ok[11:, :KEEP, :], in_=ck[11:, NEW:, :]
                ).then_inc(sem_c, 16)
                gpsimd.dma_start(
                    out=ov[11:, :KEEP, :], in_=cv[11:, NEW:, :]
                ).then_inc(sem_c, 16)

    elif variant == "stripe2":
        # Row-stripe each slice copy across the 2 HWDGE rings; every DMA has
        # outer dim 16 -> all 16 SDMA engines engaged.
        h = KEEP // 2  # 2040
        with (
            nc.semaphore("sem_a") as sem_a,
            nc.semaphore("sem_b") as sem_b,
            nc.Block() as block,
        ):

            @block.sync
            def _(sync):
                sync.dma_start(out=ok[:, KEEP:, :], in_=kn[:, :, :]).then_inc(sem_a, 16)
                sync.dma_start(
                    out=ok[:, :h, :], in_=ck[:, NEW : NEW + h, :]
                ).then_inc(sem_a, 16)
                sync.dma_start(
                    out=ov[:, :h, :], in_=cv[:, NEW : NEW + h, :]
                ).then_inc(sem_a, 16)
                sync.wait_ge(sem_a, 48)
                sync.wait_ge(sem_b, 48)

            @block.scalar
            def _(scalar):
                scalar.dma_start(out=ov[:, KEEP:, :], in_=vn[:, :, :]).then_inc(
                    sem_b, 16
                )
                scalar.dma_start(
                    out=ok[:, h:KEEP, :], in_=ck[:, NEW + h :, :]
                ).then_inc(sem_b, 16)
                scalar.dma_start(
                    out=ov[:, h:KEEP, :], in_=cv[:, NEW + h :, :]
                ).then_inc(sem_b, 16)

    elif variant == "stripe3":
        # Row-stripe each slice copy across sync HWDGE + scalar HWDGE + gpsimd
        # SWDGE; every DMA has outer dim 16 -> all 16 SDMA engines, 3 queue
        # streams per engine.
        t = KEEP // 3  # 1360
        with (
            nc.semaphore("sem_a") as sem_a,
            nc.semaphore("sem_b") as sem_b,
            nc.semaphore("sem_c") as sem_c,
            nc.Block() as block,
        ):

            @block.sync
            def _(sync):
                sync.dma_start(out=ok[:, KEEP:, :], in_=kn[:, :, :]).then_inc(sem_a, 16)
                sync.dma_start(
                    out=ok[:, :t, :], in_=ck[:, NEW : NEW + t, :]
                ).then_inc(sem_a, 16)
                sync.dma_start(
                    out=ov[:, :t, :], in_=cv[:, NEW : NEW + t, :]
                ).then_inc(sem_a, 16)
                sync.wait_ge(sem_a, 48)
                sync.wait_ge(sem_b, 48)
                sync.wait_ge(sem_c, 32)

            @block.scalar
            def _(scalar):
                scalar.dma_start(out=ov[:, KEEP:, :], in_=vn[:, :, :]).then_inc(
                    sem_b, 16
                )
                scalar.dma_start(
                    out=ok[:, t : 2 * t, :], in_=ck[:, NEW + t : NEW + 2 * t, :]
                ).then_inc(sem_b, 16)
                scalar.dma_start(
                    out=ov[:, t : 2 * t, :], in_=cv[:, NEW + t : NEW + 2 * t, :]
                ).then_inc(sem_b, 16)

            @block.gpsimd
            def _(gpsimd):
                gpsimd.dma_start(
                    out=ok[:, 2 * t : KEEP, :], in_=ck[:, NEW + 2 * t :, :]
                ).then_inc(sem_c, 16)
                gpsimd.dma_start(
                    out=ov[:, 2 * t : KEEP, :], in_=cv[:, NEW + 2 * t :, :]
                ).then_inc(sem_c, 16)

    elif variant.startswith("hybrid"):
        # sync: K rows [0:r); scalar: V rows [0:r); gpsimd: K and V rows
        # [r:KEEP). All DMAs outer dim 16 -> all 16 engines; shares roughly
        # match measured per-queue rates (HWDGE ~97 GB/s each, SWDGE ~88).
        r = int(variant.split("_")[1])
        with (
            nc.semaphore("sem_a") as sem_a,
            nc.semaphore("sem_b") as sem_b,
            nc.semaphore("sem_c") as sem_c,
            nc.Block() as block,
        ):

            @block.sync
            def _(sync):
                sync.dma_start(out=ok[:, KEEP:, :], in_=kn[:, :, :]).then_inc(sem_a, 16)
                sync.dma_start(
                    out=ok[:, :r, :], in_=ck[:, NEW : NEW + r, :]
                ).then_inc(sem_a, 16)
                sync.wait_ge(sem_a, 32)
                sync.wait_ge(sem_b, 32)
                sync.wait_ge(sem_c, 32)

            @block.scalar
            def _(scalar):
                scalar.dma_start(out=ov[:, KEEP:, :], in_=vn[:, :, :]).then_inc(
                    sem_b, 16
                )
                scalar.dma_start(
                    out=ov[:, :r, :], in_=cv[:, NEW : NEW + r, :]
                ).then_inc(sem_b, 16)

            @block.gpsimd
            def _(gpsimd):
                gpsimd.dma_start(
                    out=ok[:, r:KEEP, :], in_=ck[:, NEW + r :, :]
                ).then_inc(sem_c, 16)
                gpsimd.dma_start(
                    out=ov[:, r:KEEP, :], in_=cv[:, NEW + r :, :]
                ).then_inc(sem_c, 16)

    elif variant == "tr_tail":
        # three_rings + split each bulk copy into [big, tiny] so the final
        # completion receipt (which gates the end-of-kernel sem wait) follows
        # a small transfer instead of a big one.
        cut = KEEP - 16  # 4064 rows in the big chunk; 16-row tiny tail
        with (
            nc.semaphore("sem_a") as sem_a,
            nc.semaphore("sem_b") as sem_b,
            nc.semaphore("sem_c") as sem_c,
            nc.Block() as block,
        ):

            @block.sync
            def _(sync):
                sync.dma_start(out=ok[:, KEEP:, :], in_=kn[:, :, :]).then_inc(sem_a, 16)
                sync.dma_start(
                    out=ok[:11, :cut, :], in_=ck[:11, NEW : NEW + cut, :]
                ).then_inc(sem_a, 16)
                sync.dma_start(
                    out=ok[:11, cut:KEEP, :], in_=ck[:11, NEW + cut :, :]
                ).then_inc(sem_a, 16)
                sync.wait_ge(sem_a, 48)
                sync.wait_ge(sem_b, 48)
                sync.wait_ge(sem_c, 64)

            @block.scalar
            def _(scalar):
                scalar.dma_start(out=ov[:, KEEP:, :], in_=vn[:, :, :]).then_inc(
                    sem_b, 16
                )
                scalar.dma_start(
                    out=ov[:11, :cut, :], in_=cv[:11, NEW : NEW + cut, :]
                ).then_inc(sem_b, 16)
                scalar.dma_start(
                    out=ov[:11, cut:KEEP, :], in_=cv[:11, NEW + cut :, :]
                ).then_inc(sem_b, 16)

            @block.gpsimd
            def _(gpsimd):
                gpsimd.dma_start(
                    out=ok[11:, :cut, :], in_=ck[11:, NEW : NEW + cut, :]
                ).then_inc(sem_c, 16)
                gpsimd.dma_start(
                    out=ov[11:, :cut, :], in_=cv[11:, NEW : NEW + cut, :]
                ).then_inc(sem_c, 16)
                gpsimd.dma_start(
                    out=ok[11:, cut:KEEP, :], in_=ck[11:, NEW + cut :, :]
                ).then_inc(sem_c, 16)
                gpsimd.dma_start(
                    out=ov[11:, cut:KEEP, :], in_=cv[11:, NEW + cut :, :]
                ).then_inc(sem_c, 16)

    elif variant == "tr_tail2":
        # 11/11/10-slice split like three_rings, but gpsimd (slowest queue,
        # engine-rotating) is clipped to rows [0:3968) (64 KiB descriptors)
        # so it finishes early; the HWDGE queues carry the 112-row leftover
        # and end with tiny 16-row DMAs whose completion receipt is short.
        cut = KEEP - 16  # 4064
        clip = 3968
        with (
            nc.semaphore("sem_a") as sem_a,
            nc.semaphore("sem_b") as sem_b,
            nc.semaphore("sem_c") as sem_c,
            nc.Block() as block,
        ):

            @block.sync
            def _(sync):
                sync.dma_start(out=ok[:, KEEP:, :], in_=kn[:, :, :]).then_inc(sem_a, 16)
                sync.dma_start(
                    out=ok[:11, :cut, :], in_=ck[:11, NEW : NEW + cut, :]
                ).then_inc(sem_a, 16)
                sync.dma_start(
                    out=ok[11:, clip:KEEP, :], in_=ck[11:, NEW + clip :, :]
                ).then_inc(sem_a, 16)
                sync.dma_start(
                    out=ok[:11, cut:KEEP, :], in_=ck[:11, NEW + cut :, :]
                ).then_inc(sem_a, 16)
                sync.wait_ge(sem_a, 64)
                sync.wait_ge(sem_b, 64)
                sync.wait_ge(sem_c, 32)

            @block.scalar
            def _(scalar):
                scalar.dma_start(out=ov[:, KEEP:, :], in_=vn[:, :, :]).then_inc(
                    sem_b, 16
                )
                scalar.dma_start(
                    out=ov[:11, :cut, :], in_=cv[:11, NEW : NEW + cut, :]
                ).then_inc(sem_b, 16)
                scalar.dma_start(
                    out=ov[11:, clip:KEEP, :], in_=cv[11:, NEW + clip :, :]
                ).then_inc(sem_b, 16)
                scalar.dma_start(
                    out=ov[:11, cut:KEEP, :], in_=cv[:11, NEW + cut :, :]
                ).then_inc(sem_b, 16)

            @block.gpsimd
            def _(gpsimd):
                gpsimd.dma_start(
                    out=ok[11:, :clip, :], in_=ck[11:, NEW : NEW + clip, :]
                ).then_inc(sem_c, 16)
                gpsimd.dma_start(
                    out=ov[11:, :clip, :], in_=cv[11:, NEW : NEW + clip, :]
                ).then_inc(sem_c, 16)

    else:
        raise ValueError(variant)

    return nc


def _get_nc(variant):
    if variant not in _nc_cache:
        _nc_cache[variant] = _build_bass(variant)
    return _nc_cache[variant]


def _run(nc, inputs_by_core, trace=False, **kw):
    from concourse import bass_utils

    return bass_utils.run_bass_kernel_spmd(
        nc, inputs_by_core, core_ids=list(range(N_CORES)), trace=trace, **kw
    )


def kernel(cache_k, cache_v, k, v, _trace=False, _ret_perf=False, _variant=None, **_kw):
    variant = _variant or VARIANT
    cache_k = np.ascontiguousarray(np.asarray(cache_k, dtype=np.float32)).reshape(
        BH, MAX_LEN, D
    )
    cache_v = np.ascontiguousarray(np.asarray(cache_v, dtype=np.float32)).reshape(
        BH, MAX_LEN, D
    )
    k = np.ascontiguousarray(np.asarray(k, dtype=np.float32)).reshape(BH, NEW, D)
    v = np.ascontiguousarray(np.asarray(v, dtype=np.float32)).reshape(BH, NEW, D)

    nc = _get_nc(variant)
    in_maps = []
    for c in range(N_CORES):
        s = slice(c * SH, (c + 1) * SH)
        if variant == "merged":
            in_maps.append(
                {
                    "cache_kv": np.concatenate([cache_k[s], cache_v[s]], axis=0),
                    "kv_new": np.concatenate([k[s], v[s]], axis=0),
                }
            )
        else:
            in_maps.append(
                {"cache_k": cache_k[s], "cache_v": cache_v[s], "k": k[s], "v": v[s]}
            )

    res = _run(nc, in_maps, trace=_trace, **_kw)

    if variant == "merged":
        out_k = np.concatenate([r["out_kv"][:SH] for r in res.results], axis=0)
        out_v = np.concatenate([r["out_kv"][SH:] for r in res.results], axis=0)
    else:
        out_k = np.concatenate([r["out_k"] for r in res.results], axis=0)
        out_v = np.concatenate([r["out_v"] for r in res.results], axis=0)
    out_k = out_k.reshape(B, H, MAX_LEN, D)
    out_v = out_v.reshape(B, H, MAX_LEN, D)
    if _ret_perf:
        return (out_k, out_v), res
    return (out_k, out_v)
